# revision 9
# baseline (speedup 1.0000x reference)
"""EventSpecificTimingHeads Trainium2 kernel (8 NeuronCores, SPMD).

Shards the E=16 independent per-event attention+MLP heads across 8 cores
(2 events per core). Each core computes logits[e, b, s] for its 2 events
over the full shared feature tensor; the host gathers and transposes to
[B, S, E].

Structure per (event, batch) — all in the transposed (j, i) orientation:
  scores.T = k q.T per 32-row head group (4-way PE row tiling)
  P.T = exp(scores.T): split between ACT (exact exp) and DVE
    (Schraudolph fast exp: int16(x*128/ln2 + B) bitcast to bf16, ~3% per
    element, largely self-cancelling through softmax normalization)
  pv.T = v.T @ P.T with 4-way PE column tiling (M=32 per head)
  lsum = ones[128,32].T @ P.T -> l broadcast across each head's 32 rows
  ctx.T = pv.T * reciprocal(lsum)  (reciprocal_approx_fast + one DVE mult)
  Wo is folded into W1 on the host (W1' = W1 @ Wo), so:
  h1 = relu(W1' ctx + c1), two (ev,b) packed per PSUM bank (M=64 col tiles)
  logits = w2.T @ h1 + b2, two (ev,b) packed per bank (M=1 col tiles)
QKV biases are folded in as K=1 rank-1 matmul accumulations so every
psum drain is a pure copy, freely assignable to ACT or DVE. The MLP tail
(relu/W2/logits) for a pair is emitted one iteration late so its psum
dependencies are long-resolved when ACT/DVE reach those instructions.
"""
import sys

if "/opt/trn_rl_repo" not in sys.path:
    sys.path.insert(0, "/opt/trn_rl_repo")

import numpy as np
import ml_dtypes

import concourse.bass as bass
import concourse.bacc as bacc
import concourse.tile as tile
from concourse import mybir
from concourse.bass_utils import run_bass_kernel_spmd

BF16 = mybir.dt.bfloat16
F32 = mybir.dt.float32
I16 = mybir.dt.int16
AF = mybir.ActivationFunctionType
ALU = mybir.AluOpType

E, D, B, S, H, Dh, H2 = 16, 128, 8, 512, 4, 32, 64
T = B * S            # 4096
EV = 2               # events per core
NCORES = 8

# Schraudolph fast-exp constants (bf16 target, trunc-compensated)
EXP_A = 128.0 / float(np.log(2.0))     # 184.664965
EXP_B = 16256.0 - 5.59 + 0.5

# Per-(ev,b) exp engine split: tile k (of 8 [128,1024] tiles) -> DVE if in set
DVE_TILES = (2, 5)

_CACHED_NC = None


def build_nc():
    nc = bacc.Bacc(None, target_bir_lowering=False, debug=False)

    xT_d = nc.declare_dram_parameter("xT", [D, T], BF16, isOutput=False)
    wqkT_d = nc.declare_dram_parameter("wqkT", [D, EV, 2, D], BF16, isOutput=False)
    wvT_d = nc.declare_dram_parameter("wvT", [D, EV, D], BF16, isOutput=False)
    bqkT_d = nc.declare_dram_parameter("bqkT", [1, EV, 2, D], BF16, isOutput=False)
    w1pT_d = nc.declare_dram_parameter("w1pT", [D, EV, H2], BF16, isOutput=False)
    c1b2_d = nc.declare_dram_parameter("c1b2", [D, EV], F32, isOutput=False)
    w2dup_d = nc.declare_dram_parameter("w2dup", [D, EV, 32], BF16, isOutput=False)
    b2rep_d = nc.declare_dram_parameter("b2rep", [D, EV], F32, isOutput=False)
    out_d = nc.declare_dram_parameter("out", [EV, B, S], F32, isOutput=True)

    with tile.TileContext(nc) as tc:
        with (
            tc.tile_pool(name="single", bufs=1) as single,
            tc.tile_pool(name="work", bufs=2) as work,
            tc.tile_pool(name="ps", bufs=1, space="PSUM") as psp,
        ):
            # ---- resident SBUF tensors ----
            xT_sb = single.tile([D, T], BF16)
            wqkT_sb = single.tile([D, EV, 2, D], BF16)
            wvT_sb = single.tile([D, EV, D], BF16)
            bqkT_sb = single.tile([1, EV, 2, D], BF16)
            w1pT_sb = single.tile([D, EV, H2], BF16)
            c1b2_sb = single.tile([D, EV], F32)
            w2dup_sb = single.tile([D, EV, 32], BF16)
            b2rep_sb = single.tile([D, EV], F32)
            ones32 = single.tile([D, 32], BF16)
            ones1 = single.tile([1, S], BF16)
            # q/k (bf16, bias folded in): [d, ev, b, {q,k}, s-in-chunk]
            qkT_sb = single.tile([D, EV, B, 2, S], BF16)
            # v: [j-in-chunk, b, jc, ev, h, dh]
            v_sb = single.tile([D, B, 4, EV, H, Dh], BF16)

            nc.sync.dma_start(out=wqkT_sb[:], in_=wqkT_d[:])
            nc.sync.dma_start(out=bqkT_sb[:], in_=bqkT_d[:])
            nc.sync.dma_start(out=wvT_sb[:], in_=wvT_d[:])
            for n in range(8):
                nc.scalar.dma_start(out=xT_sb[:, n * S:(n + 1) * S],
                                    in_=xT_d[:, n * S:(n + 1) * S])
            nc.sync.dma_start(out=w1pT_sb[:], in_=w1pT_d[:])
            nc.sync.dma_start(out=c1b2_sb[:], in_=c1b2_d[:])
            nc.sync.dma_start(out=w2dup_sb[:], in_=w2dup_d[:])
            nc.sync.dma_start(out=b2rep_sb[:], in_=b2rep_d[:])
            nc.gpsimd.memset(ones32[:], 1.0)
            nc.gpsimd.memset(ones1[:], 1.0)

            def proj_qk(eb):
                """q,k projection for one (ev, chunk b); bias via K=1 matmul,
                drain is a pure copy (engine alternates by eb parity)."""
                ev, b = eb // B, eb % B
                t0 = b * S
                ps = psp.tile([D, 2, S], F32, name="proj", tag="st", bufs=2)
                for qk in range(2):
                    nc.tensor.matmul(
                        ps[:, qk, :],
                        wqkT_sb[:, ev, qk, :],
                        xT_sb[:, t0:t0 + S],
                        start=True, stop=False,
                    )
                    nc.tensor.matmul(
                        ps[:, qk, :],
                        bqkT_sb[0:1, ev, qk, :],
                        ones1[:],
                        start=False, stop=True,
                    )
                if eb % 2 == 0:
                    nc.vector.tensor_copy(qkT_sb[:, ev, b, :, :], ps[:])
                else:
                    nc.scalar.activation(qkT_sb[:, ev, b, :, :], ps[:], AF.Copy)

            def project_v(b):
                """v for both events of chunk-group b; pure-copy drain."""
                psv = psp.tile([D, 2, S], F32, name="psv", tag="st", bufs=2)
                for c in range(4):
                    tch = 4 * b + c
                    nc.tensor.matmul(
                        psv[:, c // 2, (c % 2) * 256:(c % 2) * 256 + 256],
                        xT_sb[:, tch * D:(tch + 1) * D],
                        wvT_sb[:].rearrange("p e d -> p (e d)"),
                    )
                nc.vector.tensor_copy(
                    v_sb[:, b],
                    psv[:].rearrange("p a (f e h d) -> p (a f) e h d",
                                     f=2, e=EV, h=H),
                )

            def mlp_tail(p, mlp, h1_sb):
                """relu + 2x W2 + logits for pair p = (2p, 2p+1). Emitted one
                iteration after W1(2p+1) so deps are resolved by then."""
                ev = (2 * p) // B
                bb = (2 * p) % B
                nc.scalar.activation(
                    h1_sb[:], mlp[:, 0, :], AF.Relu,
                    bias=c1b2_sb[:, ev:ev + 1],
                )
                for j in range(2):
                    hh = 64 * j
                    nc.tensor.matmul(
                        mlp[32 * j:32 * j + 32, 1, :],
                        w2dup_sb[hh:hh + 64, ev, :],
                        h1_sb[hh:hh + 64, :],
                        tile_position=(hh, 32 * j),
                    )
                lg = work.tile([33, S], F32, name="lg", tag="lg")
                nc.scalar.activation(
                    lg[:], mlp[0:33, 1, :], AF.Identity,
                    bias=b2rep_sb[0:33, ev:ev + 1],
                )
                nc.sync.dma_start(
                    out=out_d[ev, bb:bb + 2, :],
                    in_=lg[0:33:32, :],
                )

            # ---- prologue ----
            project_v(0)
            proj_qk(0)
            proj_qk(1)

            mlp = None
            h1_sb = None
            pending_tail = None
            for eb in range(EV * B):
                ev, b = eb // B, eb % B
                # QK^T + exp, one [128, 2*S] psum tile per (jc, head-pair)
                pt = work.tile([D, 4, H, S], BF16, name="pt", tag="pt")
                for k in range(8):
                    jc, hp = k // 2, k % 2
                    st = psp.tile([D, 2, S], F32, name="st", tag="st", bufs=2)
                    for h2 in range(2):
                        h = 2 * hp + h2
                        nc.tensor.matmul(
                            st[:, h2, :],
                            qkT_sb[32 * h:32 * h + 32, ev, b, 1,
                                   jc * D:(jc + 1) * D],
                            qkT_sb[32 * h:32 * h + 32, ev, b, 0, :],
                            tile_position=(32 * h, 0),
                        )
                    dst = pt[:, jc, 2 * hp:2 * hp + 2, :]
                    if k in DVE_TILES:
                        nc.vector.tensor_scalar(
                            dst.bitcast(I16), st[:], EXP_A, EXP_B,
                            ALU.mult, ALU.add,
                        )
                    else:
                        nc.scalar.activation(dst, st[:], AF.Exp)
                # deferred MLP tail of the previous pair (deps long done)
                if pending_tail is not None:
                    mlp_tail(*pending_tail)
                    pending_tail = None
                # PV + broadcast row-sums, 4-way column tiling
                pvl = psp.tile([D, 2, S], F32, name="pvl", tag="pvl")
                for h in range(H):
                    for jc in range(4):
                        nc.tensor.matmul(
                            pvl[32 * h:32 * h + 32, 0, :],
                            v_sb[:, b, jc, ev, h, :],
                            pt[:, jc, h, :],
                            start=(jc == 0), stop=(jc == 3),
                            tile_position=(0, 32 * h),
                        )
                    for jc in range(4):
                        nc.tensor.matmul(
                            pvl[32 * h:32 * h + 32, 1, :],
                            ones32[:],
                            pt[:, jc, h, :],
                            start=(jc == 0), stop=(jc == 3),
                            tile_position=(0, 32 * h),
                        )
                linv = work.tile([D, S], F32, name="linv", tag="linv")
                nc.vector.reciprocal_approx_fast(out=linv[:], in_=pvl[:, 1, :])
                ctxT = work.tile([D, S], BF16, name="ctxT", tag="ctxT")
                nc.vector.tensor_tensor(ctxT[:], pvl[:, 0, :], linv[:], ALU.mult)
                # W1' = W1 @ Wo; two (ev,b) per psum bank via column tiling
                if eb % 2 == 0:
                    mlp = psp.tile([D, 2, S], F32, name="mlp", tag="mlp")
                half = 64 * (eb % 2)
                nc.tensor.matmul(
                    mlp[half:half + 64, 0, :],
                    w1pT_sb[:, ev, :],
                    ctxT[:],
                    tile_position=(0, half),
                )
                if eb % 2 == 1:
                    h1_sb = work.tile([D, S], BF16, name="h1", tag="h1")
                    pending_tail = (eb // 2, mlp, h1_sb)
                # prefetch next projections (placed late so their psum-drain
                # ops land after this iteration's exp work in engine order)
                if eb + 2 < EV * B:
                    proj_qk(eb + 2)
                if b + 1 < B and ev == 0:
                    project_v(b + 1)
            mlp_tail(*pending_tail)

    nc.compile()
    return nc


def _prep_inputs(lstm_features, Wqkv, bqkv, Wo, bo, W1, b1, W2, b2):
    """Host-side per-core input prep (numpy, fp32 -> bf16 where PE-facing)."""
    bf = ml_dtypes.bfloat16
    x = np.asarray(lstm_features, np.float32).reshape(T, D)
    xT = np.ascontiguousarray(x.T).astype(bf)
    scale = 1.0 / np.sqrt(np.float32(Dh))

    in_maps = []
    for c in range(NCORES):
        evs = [2 * c, 2 * c + 1]
        wqkT = np.zeros((D, EV, 2, D), np.float32)
        bqkT = np.zeros((1, EV, 2, D), np.float32)
        wvT = np.zeros((D, EV, D), np.float32)
        w1pT = np.zeros((D, EV, H2), np.float32)
        c1b2 = np.zeros((D, EV), np.float32)
        w2dup = np.zeros((D, EV, 32), np.float32)
        b2rep = np.zeros((D, EV), np.float32)
        for i, e in enumerate(evs):
            Wq = Wqkv[e, 0:D, :] * scale
            Wk = Wqkv[e, D:2 * D, :]
            Wv = Wqkv[e, 2 * D:3 * D, :]
            wqkT[:, i, 0, :] = Wq.T
            wqkT[:, i, 1, :] = Wk.T
            wvT[:, i, :] = Wv.T
            bqkT[0, i, 0, :] = bqkv[e, 0:D] * scale
            bqkT[0, i, 1, :] = bqkv[e, D:2 * D]
            bv = bqkv[e, 2 * D:3 * D]
            bo_eff = Wo[e] @ bv + bo[e]
            W1p = W1[e] @ Wo[e]           # [H2, D]
            w1pT[:, i, :] = W1p.T
            c1 = W1[e] @ bo_eff + b1[e]   # [H2]
            c1b2[0:H2, i] = c1
            c1b2[H2:D, i] = c1
            w2dup[0:H2, i, 0] = W2[e, 0, :]
            w2dup[H2:D, i, 0] = W2[e, 0, :]
            b2rep[:, i] = b2[e, 0]
        in_maps.append({
            "xT": xT,
            "wqkT": wqkT.astype(bf),
            "wvT": wvT.astype(bf),
            "bqkT": bqkT.astype(bf),
            "w1pT": w1pT.astype(bf),
            "c1b2": c1b2,
            "w2dup": w2dup.astype(bf),
            "b2rep": b2rep,
        })
    return in_maps


def kernel(lstm_features, Wqkv, bqkv, Wo, bo, W1, b1, W2, b2, _trace=False):
    global _CACHED_NC
    args = [np.asarray(a, np.float32) for a in
            (lstm_features, Wqkv, bqkv, Wo, bo, W1, b1, W2, b2)]
    in_maps = _prep_inputs(*args)
    if _CACHED_NC is None:
        _CACHED_NC = build_nc()
    res = run_bass_kernel_spmd(
        _CACHED_NC, in_maps, list(range(NCORES)), trace=_trace
    )
    logits = np.concatenate(
        [np.asarray(res.results[c]["out"], np.float32) for c in range(NCORES)],
        axis=0,
    )  # [16, 8, 512]
    out = np.ascontiguousarray(logits.transpose(1, 2, 0))  # [B, S, E]
    if _trace:
        return out, res
    return out
